# revision 1
# baseline (speedup 1.0000x reference)
"""Compressed Interaction Network (CIN) forward on 8 Trainium2 NeuronCores.

Math (per batch item, m=32 fields, d=64 embed, H=256 hidden):
    x0 = x[i]                          # (m, d)
    h  = x0
    layer l in 0..2:
        z = outer(x0, h) over d        # (m*n, d), z[(a,b),:] = x0[a,:]*h[b,:]
        y = relu(W_l^T z + b_l)        # (H, d)
        xcur, h = split_half(y) (layers 0,1); xcur = h = y (layer 2)
    f = concat(xcur_0, xcur_1, xcur_2) # (512, d)
    out[i] = sum_d(f) @ fc_W + fc_b    # scalar

Mapping: batch 1024 -> 8 cores x 128 items, 16 groups of 8 items per core.
 - Outer-product operands are built with DMA broadcast reads (stride-0 APs),
   one DMA per tile so consumers wait on a single DMA semaphore lane.
 - z tiles computed on VectorE in fp16 (2x mode), layout [k-part, (i, m, d)].
 - Conv matmuls on PE: stationary W chunks [128, 128] fp16, moving z
   [128, 512] (8 items x 64 d), accumulated over k-chunks in fp32 PSUM.
 - Bias+ReLU fused into the PSUM->SBUF move on ScalarE; per-item d-sums for
   the final FC are fused there too via accum_out.
 - Final dot: PE matmul of [128,1] fc weight chunks against [128, 128] sums.
"""

import numpy as np

import concourse.bass as bass
import concourse.tile as tile
from concourse import mybir
from concourse.bass_utils import run_bass_kernel_spmd

N_CORES = 8
B_TOTAL = 1024
B_CORE = B_TOTAL // N_CORES  # 128
M = 32  # num fields
D = 64  # embed dim
H = 256  # conv output channels
GROUP = 8  # items per group (512 moving columns)
N_GROUPS = B_CORE // GROUP  # 16
MD = M * D  # 2048, elements per item row

F16 = mybir.dt.float16
F32 = mybir.dt.float32
RELU = mybir.ActivationFunctionType.Relu
IDENT = mybir.ActivationFunctionType.Identity


def build():
    nc = bass.Bass()
    xh = nc.declare_dram_parameter("xh", [B_CORE, M, D], F16, isOutput=False)
    # x rows tiled 4x along the field axis: xr[i, p, d] = x[i, p % 32, d]
    xr = nc.declare_dram_parameter("xr", [B_CORE, 128, D], F16, isOutput=False)
    w0 = nc.declare_dram_parameter("w0", [8, 128, H], F16, isOutput=False)
    w1 = nc.declare_dram_parameter("w1", [32, 128, H], F16, isOutput=False)
    w2 = nc.declare_dram_parameter("w2", [32, 128, H], F16, isOutput=False)
    bia = nc.declare_dram_parameter("bia", [128, 3, 2], F32, isOutput=False)
    fcw = nc.declare_dram_parameter("fcw", [128, 4], F32, isOutput=False)
    fcb = nc.declare_dram_parameter("fcb", [1, 1], F32, isOutput=False)
    out = nc.declare_dram_parameter("out", [B_CORE, 1], F32, isOutput=True)

    with tile.TileContext(nc) as tc:
        with (
            tc.tile_pool(name="consts", bufs=1) as consts,
            tc.tile_pool(name="bpool", bufs=2) as bpool,
            tc.tile_pool(name="epool", bufs=4) as epool,
            tc.tile_pool(name="zpool", bufs=3) as zpool,
            tc.tile_pool(name="hpool", bufs=4) as hpool,
            tc.tile_pool(name="spool", bufs=1) as spool,
            tc.tile_pool(name="ppool", bufs=6, space="PSUM") as ppool,
            tc.tile_pool(name="fcp", bufs=1, space="PSUM") as fcp,
        ):
            # --- resident constants ---
            w0_sb = consts.tile([128, 8, H], F16, tag="w0")
            nc.sync.dma_start(w0_sb[:], w0[:].rearrange("c k o -> k c o"))
            w1_sb = consts.tile([128, 32, H], F16, tag="w1")
            nc.sync.dma_start(w1_sb[:], w1[:].rearrange("c k o -> k c o"))
            w2_sb = consts.tile([128, 32, H], F16, tag="w2")
            nc.sync.dma_start(w2_sb[:], w2[:].rearrange("c k o -> k c o"))
            bia_sb = consts.tile([128, 3, 2], F32, tag="bia")
            nc.sync.dma_start(bia_sb[:], bia[:])
            fcw_sb = consts.tile([128, 4], F32, tag="fcw")
            nc.sync.dma_start(fcw_sb[:], fcw[:])
            fcb_sb = consts.tile([1, 1], F32, tag="fcb")
            nc.sync.dma_start(fcb_sb[:], fcb[:])

            # per-item d-sums of the relu'd xs channels, [channel, item]
            s_tiles = [
                spool.tile([128, B_CORE], F32, tag=f"s{c}", name=f"s{c}")
                for c in range(4)
            ]

            for g in range(N_GROUPS):
                i0 = g * GROUP

                # B[p, i, m, d] = x_i[m, d] for every partition p
                # (one DMA: partition loop stride 0, (m d) merged contiguous)
                Bg = bpool.tile([128, GROUP, M, D], F16, tag="B")
                src = bass.AP(
                    tensor=xh,
                    offset=i0 * MD,
                    ap=[[0, 128], [MD, GROUP], [1, MD]],
                )
                nc.sync.dma_start(Bg[:], src)

                # R[p, i, d] = x_i[p % 32, d]  (from the host-tiled copy)
                Rg = epool.tile([128, GROUP, D], F16, tag="R")
                src = bass.AP(
                    tensor=xr,
                    offset=i0 * 128 * D,
                    ap=[[D, 128], [128 * D, GROUP], [1, D]],
                )
                nc.sync.dma_start(Rg[:], src)

                # ---------- layer 0: z0[(a,b)] = x[4c + p//32] * x[p%32] ----------
                # per 32-partition block s: z0[32s:32s+32] = R-block * B[:, :, 4c+s]
                # (operands of an engine op must share the partition range)
                ps0 = [
                    ppool.tile([128, GROUP * D], F32, tag="yps", name="ps0")
                    for _ in range(2)
                ]
                for c in range(8):
                    z0 = epool.tile([128, GROUP, D], F16, tag="z0")
                    for s in range(4):
                        pr = slice(32 * s, 32 * (s + 1))
                        nc.vector.tensor_mul(
                            z0[pr, :, :],
                            Rg[pr, :, :],
                            Bg[pr, :, 4 * c + s, :],
                        )
                    for oc in range(2):
                        nc.tensor.matmul(
                            ps0[oc][:],
                            w0_sb[:, c, oc * 128 : (oc + 1) * 128],
                            z0[:],
                            start=(c == 0),
                            stop=(c == 7),
                        )

                # psum -> sbuf with fused bias+relu; chunk1 becomes next h,
                # chunk0 only needs its per-item d-sums (accum_out into s0)
                h1 = hpool.tile([128, GROUP, D], F16, tag="h1")
                nc.scalar.activation(h1[:], ps0[1][:], RELU, bias=bia_sb[:, 0, 1:2])
                for i in range(GROUP):
                    sl = slice(i * D, (i + 1) * D)
                    nc.scalar.activation(
                        ps0[0][:, sl],
                        ps0[0][:, sl],
                        RELU,
                        bias=bia_sb[:, 0, 0:1],
                        accum_out=s_tiles[0][:, i0 + i : i0 + i + 1],
                    )

                # ---------- layers 1 and 2 ----------
                for lay in range(2):
                    w_sb = w1_sb if lay == 0 else w2_sb
                    h_in = h1 if lay == 0 else h2
                    ps = [
                        ppool.tile([128, GROUP * D], F32, tag="yps", name="ps")
                        for _ in range(2)
                    ]
                    for mb in range(8):
                        zt = zpool.tile([128, GROUP, 4, D], F16, tag="z")
                        nc.vector.tensor_mul(
                            zt[:],
                            h_in[:, :, None, :].to_broadcast((128, GROUP, 4, D)),
                            Bg[:, :, 4 * mb : 4 * mb + 4, :],
                        )
                        for mm in range(4):
                            m = 4 * mb + mm
                            for oc in range(2):
                                nc.tensor.matmul(
                                    ps[oc][:],
                                    w_sb[:, m, oc * 128 : (oc + 1) * 128],
                                    zt[:, :, mm, :],
                                    start=(m == 0),
                                    stop=(m == 31),
                                )
                    if lay == 0:
                        # split_half: chunk0 -> s1 sums, chunk1 -> h2
                        h2 = hpool.tile([128, GROUP, D], F16, tag="h2")
                        nc.scalar.activation(
                            h2[:], ps[1][:], RELU, bias=bia_sb[:, 1, 1:2]
                        )
                        for i in range(GROUP):
                            sl = slice(i * D, (i + 1) * D)
                            nc.scalar.activation(
                                ps[0][:, sl],
                                ps[0][:, sl],
                                RELU,
                                bias=bia_sb[:, 1, 0:1],
                                accum_out=s_tiles[1][:, i0 + i : i0 + i + 1],
                            )
                    else:
                        # last layer: both chunks feed the FC sums (s2, s3)
                        for oc in range(2):
                            for i in range(GROUP):
                                sl = slice(i * D, (i + 1) * D)
                                nc.scalar.activation(
                                    ps[oc][:, sl],
                                    ps[oc][:, sl],
                                    RELU,
                                    bias=bia_sb[:, 2, oc : oc + 1],
                                    accum_out=s_tiles[2 + oc][
                                        :, i0 + i : i0 + i + 1
                                    ],
                                )

            # ---------- final FC: out[i] = sum_c fcw[c] * s[c, i] + fcb ----------
            fc_ps = fcp.tile([1, B_CORE], F32, tag="fc")
            for c in range(4):
                nc.tensor.matmul(
                    fc_ps[:],
                    fcw_sb[:, c : c + 1],
                    s_tiles[c][:],
                    start=(c == 0),
                    stop=(c == 3),
                )
            osb = consts.tile([1, B_CORE], F32, tag="osb")
            nc.scalar.activation(osb[:], fc_ps[:], IDENT, bias=fcb_sb[0:1, 0:1])
            nc.sync.dma_start(out[:], osb[:])

    _legalize_waits(nc)
    return nc


def _legalize_waits(nc, max_waits=1):
    """walrus codegen allows at most 2 semaphore waits per instruction; spill
    the excess onto NoOps injected just before the offender on the same
    engine (same-engine FIFO makes this ordering-equivalent)."""
    for bb in nc.main_func.blocks:
        insts = bb.instructions
        i = 0
        new_list = []
        changed = False
        for ins in insts:
            si = ins.sync_info
            if si is not None and si.on_wait and len(si.on_wait) > max_waits:
                waits = list(si.on_wait)
                extra, keep = waits[:-max_waits], waits[-max_waits:]
                k = 0
                while k < len(extra):
                    chunk = extra[k : k + max_waits]
                    nop = mybir.InstNoOp(name=f"{ins.name}-w{k}", ins=[], outs=[])
                    nop.engine = ins.engine
                    nop.sync_info = mybir.SyncInfo(on_wait=chunk, on_update=[])
                    new_list.append(nop)
                    k += max_waits
                ins.sync_info = mybir.SyncInfo(
                    on_wait=keep,
                    on_update=list(si.on_update) if si.on_update else [],
                )
                changed = True
            new_list.append(ins)
        if changed:
            if hasattr(bb, "set_instructions"):
                bb.set_instructions(new_list)
            else:
                insts.clear()
                insts.extend(new_list)
                if len(bb.instructions) != len(new_list):
                    bb.instructions = new_list


def prep_inputs(x, W0, b0, W1, b1, W2, b2, fc_W, fc_b):
    """Host-side reshape/cast into the per-core input maps."""
    xh = np.ascontiguousarray(x.astype(np.float16))
    xr = np.ascontiguousarray(
        np.tile(xh.reshape(B_TOTAL, 1, M, D), (1, 4, 1, 1)).reshape(
            B_TOTAL, 128, D
        )
    )
    w0 = np.ascontiguousarray(W0.astype(np.float16).reshape(8, 128, H))
    w1 = np.ascontiguousarray(W1.astype(np.float16).reshape(32, 128, H))
    w2 = np.ascontiguousarray(W2.astype(np.float16).reshape(32, 128, H))
    bia = np.ascontiguousarray(
        np.stack([b0, b1, b2]).reshape(3, 2, 128).transpose(2, 0, 1).astype(np.float32)
    )
    fcw = np.ascontiguousarray(fc_W.reshape(4, 128).T.astype(np.float32))
    fcb = np.ascontiguousarray(fc_b.reshape(1, 1).astype(np.float32))
    shared = {"w0": w0, "w1": w1, "w2": w2, "bia": bia, "fcw": fcw, "fcb": fcb}
    return [
        {
            "xh": xh[i * B_CORE : (i + 1) * B_CORE],
            "xr": xr[i * B_CORE : (i + 1) * B_CORE],
            **shared,
        }
        for i in range(N_CORES)
    ]


_NC = None


def _get_nc():
    global _NC
    if _NC is None:
        _NC = build()
    return _NC


def kernel(**inputs):
    in_maps = prep_inputs(**inputs)
    res = run_bass_kernel_spmd(_get_nc(), in_maps, list(range(N_CORES)))
    return np.ascontiguousarray(
        np.concatenate([r["out"] for r in res.results], axis=0).astype(np.float32)
    )



# revision 2
# speedup vs baseline: 1.3836x; 1.3836x over previous
"""Compressed Interaction Network (CIN) forward on 8 Trainium2 NeuronCores.

Math (per batch item, m=32 fields, d=64 embed, H=256 hidden):
    x0 = x[i]                          # (m, d)
    h  = x0
    layer l in 0..2:
        z = outer(x0, h) over d        # (m*n, d), z[(a,b),:] = x0[a,:]*h[b,:]
        y = relu(W_l^T z + b_l)        # (H, d)
        xcur, h = split_half(y) (layers 0,1); xcur = h = y (layer 2)
    f = concat(xcur_0, xcur_1, xcur_2) # (512, d)
    out[i] = sum_d(f) @ fc_W + fc_b    # scalar

Mapping: batch 1024 -> 8 cores x 128 items, 16 groups of 8 items per core.

v2 design notes (vs the earlier per-group-serial version):
 - Layer 0 exploits z0 symmetry: z0[(a,b)] = z0[(b,a)], so W0 is folded on
   the host to 528 = 32*33/2 unique pair rows (padded to 640 = 5 k-chunks
   of 128).  The two z0 operands are host-gathered tensors (xqa, xqb) laid
   out so ONE full-128-partition DVE multiply builds a whole group's z0.
 - Layers 1/2: z chunks built on VectorE in fp16 (2x mode) from h x Bg
   (x broadcast to all partitions by one DMA per group); each chunk is
   consumed immediately by two interleaved matmuls (oc0/oc1) so at most a
   couple of chunks are alive.
 - Software pipeline over groups keeps the PE queue dense (HAM stays at
   K=8/8).  PE order per iteration i: [L1(i)][L0(i+1)][L2(i)].  The ACT of
   h2(i) + DVE build of z2(i) hide under L0(i+1); ACT h1(i+1) + build of
   z1(i+1) hide under L2(i).
 - Bias+ReLU fused in the PSUM->SBUF ACT; per-item d-sums for the final FC
   are 4 DVE X-axis reduces per group (fp32 out) instead of 512 per-item
   ScalarE accum ops.
 - Final dot: PE matmul of [128,1] fc weight chunks against [128,128] sums.
"""

import numpy as np

import concourse.bass as bass
import concourse.tile as tile
from concourse import mybir
from concourse.bass_utils import run_bass_kernel_spmd

N_CORES = 8
B_TOTAL = 1024
B_CORE = B_TOTAL // N_CORES  # 128
M = 32  # num fields
D = 64  # embed dim
H = 256  # conv output channels
GROUP = 8  # items per group (512 moving columns)
N_GROUPS = B_CORE // GROUP  # 16
MD = M * D  # 2048, elements per item row
NP0 = (M * (M + 1)) // 2  # 528 unique symmetric pairs in layer 0
C0 = (NP0 + 127) // 128  # 5 k-chunks of 128 (padded with zero weight rows)

F16 = mybir.dt.float16
F32 = mybir.dt.float32
RELU = mybir.ActivationFunctionType.Relu
IDENT = mybir.ActivationFunctionType.Identity
AX_X = mybir.AxisListType.X


def build():
    nc = bass.Bass()
    xh = nc.declare_dram_parameter("xh", [B_CORE, M, D], F16, isOutput=False)
    # layer-0 symmetric-pair operands: xqa[i, p, c, d] = x_i[amap[c*128+p], d]
    xqa = nc.declare_dram_parameter("xqa", [B_CORE, 128, C0, D], F16, isOutput=False)
    xqb = nc.declare_dram_parameter("xqb", [B_CORE, 128, C0, D], F16, isOutput=False)
    w0s = nc.declare_dram_parameter("w0s", [C0, 128, H], F16, isOutput=False)
    w1 = nc.declare_dram_parameter("w1", [32, 128, H], F16, isOutput=False)
    w2 = nc.declare_dram_parameter("w2", [32, 128, H], F16, isOutput=False)
    bia = nc.declare_dram_parameter("bia", [128, 3, 2], F32, isOutput=False)
    fcw = nc.declare_dram_parameter("fcw", [128, 4], F32, isOutput=False)
    fcb = nc.declare_dram_parameter("fcb", [1, 1], F32, isOutput=False)
    out = nc.declare_dram_parameter("out", [B_CORE, 1], F32, isOutput=True)

    with tile.TileContext(nc) as tc:
        with (
            tc.tile_pool(name="consts", bufs=1) as consts,
            tc.tile_pool(name="bgpool", bufs=2) as bgpool,
            tc.tile_pool(name="xqpool", bufs=2) as xqpool,
            tc.tile_pool(name="z0pool", bufs=3) as z0pool,
            tc.tile_pool(name="zpool", bufs=12) as zpool,
            tc.tile_pool(name="hpool", bufs=4) as hpool,
            tc.tile_pool(name="rypool", bufs=6) as rypool,
            tc.tile_pool(name="spool", bufs=1) as spool,
            tc.tile_pool(name="ppool", bufs=6, space="PSUM") as ppool,
            tc.tile_pool(name="fcp", bufs=1, space="PSUM") as fcp,
        ):
            # --- resident constants ---
            w0s_sb = consts.tile([128, C0, H], F16, tag="w0s")
            nc.sync.dma_start(w0s_sb[:], w0s[:].rearrange("c k o -> k c o"))
            w1_sb = consts.tile([128, 32, H], F16, tag="w1")
            nc.sync.dma_start(w1_sb[:], w1[:].rearrange("c k o -> k c o"))
            w2_sb = consts.tile([128, 32, H], F16, tag="w2")
            nc.sync.dma_start(w2_sb[:], w2[:].rearrange("c k o -> k c o"))
            bia_sb = consts.tile([128, 3, 2], F32, tag="bia")
            nc.sync.dma_start(bia_sb[:], bia[:])
            fcw_sb = consts.tile([128, 4], F32, tag="fcw")
            nc.sync.dma_start(fcw_sb[:], fcw[:])
            fcb_sb = consts.tile([1, 1], F32, tag="fcb")
            nc.sync.dma_start(fcb_sb[:], fcb[:])

            # per-item d-sums of the relu'd s-half channels, [chan, chunk, item]
            s_sb = spool.tile([128, 4, B_CORE], F32, tag="s")

            # pipeline state (python-side references to live tiles)
            bg_t, xqa_t, xqb_t = {}, {}, {}
            z0_t, z1_t, z2_t = {}, {}, {}
            h1_t, h2_t = {}, {}
            ry_t = {}  # (g, chunk) -> tile; chunks 0..3 = L0oc0, L1oc0, L2oc0, L2oc1
            ps01_t, ps12_t = {}, {}  # psum pairs for L0, and for L1/L2

            def dma_xq(g):
                i0 = g * GROUP
                for name, src, dst_map in (("xqa", xqa, xqa_t), ("xqb", xqb, xqb_t)):
                    t = xqpool.tile([128, C0, GROUP, D], F16, tag=name)
                    ap = bass.AP(
                        tensor=src,
                        offset=i0 * 128 * C0 * D,
                        ap=[[C0 * D, 128], [D, C0], [128 * C0 * D, GROUP], [1, D]],
                    )
                    nc.sync.dma_start(t[:], ap)
                    dst_map[g] = t

            def dma_bg(g):
                i0 = g * GROUP
                t = bgpool.tile([128, GROUP, M, D], F16, tag="B")
                src = bass.AP(
                    tensor=xh,
                    offset=i0 * MD,
                    ap=[[0, 128], [MD, GROUP], [1, MD]],
                )
                nc.sync.dma_start(t[:], src)
                bg_t[g] = t

            def build_z0(g):
                t = z0pool.tile([128, C0, GROUP, D], F16, tag="z0")
                nc.vector.tensor_mul(t[:], xqa_t[g][:], xqb_t[g][:])
                z0_t[g] = t
                del xqa_t[g], xqb_t[g]

            def mm_l0(g):
                ps = [
                    ppool.tile([128, GROUP * D], F32, tag="ps", name="ps0")
                    for _ in range(2)
                ]
                for c in range(C0):
                    for oc in range(2):
                        nc.tensor.matmul(
                            ps[oc][:],
                            w0s_sb[:, c, oc * 128 : (oc + 1) * 128],
                            z0_t[g][:, c, :, :],
                            start=(c == 0),
                            stop=(c == C0 - 1),
                        )
                ps01_t[g] = ps
                del z0_t[g]

            def act_l0(g):
                ps = ps01_t[g]
                h = hpool.tile([128, GROUP, D], F16, tag="h1")
                nc.scalar.activation(h[:], ps[1][:], RELU, bias=bia_sb[:, 0, 1:2])
                h1_t[g] = h
                r = rypool.tile([128, GROUP, D], F16, tag="ry")
                nc.scalar.activation(r[:], ps[0][:], RELU, bias=bia_sb[:, 0, 0:1])
                ry_t[(g, 0)] = r
                del ps01_t[g]

            def build_z12(g, lay):
                h = h1_t[g] if lay == 1 else h2_t[g]
                tiles = []
                for mb in range(8):
                    zt = zpool.tile([128, GROUP, 4, D], F16, tag="z")
                    nc.vector.tensor_mul(
                        zt[:],
                        h[:, :, None, :].to_broadcast((128, GROUP, 4, D)),
                        bg_t[g][:, :, 4 * mb : 4 * mb + 4, :],
                    )
                    tiles.append(zt)
                if lay == 1:
                    z1_t[g] = tiles
                    del h1_t[g]
                else:
                    z2_t[g] = tiles
                    del h2_t[g]

            def mm_l12(g, lay):
                w_sb = w1_sb if lay == 1 else w2_sb
                tiles = z1_t[g] if lay == 1 else z2_t[g]
                ps = [
                    ppool.tile([128, GROUP * D], F32, tag="ps", name="ps12")
                    for _ in range(2)
                ]
                for mb in range(8):
                    for mm in range(4):
                        m = 4 * mb + mm
                        for oc in range(2):
                            nc.tensor.matmul(
                                ps[oc][:],
                                w_sb[:, m, oc * 128 : (oc + 1) * 128],
                                tiles[mb][:, :, mm, :],
                                start=(m == 0),
                                stop=(m == 31),
                            )
                ps12_t[g] = ps
                if lay == 1:
                    del z1_t[g]
                else:
                    del z2_t[g]

            def act_l1(g):
                ps = ps12_t[g]
                h = hpool.tile([128, GROUP, D], F16, tag="h2")
                nc.scalar.activation(h[:], ps[1][:], RELU, bias=bia_sb[:, 1, 1:2])
                h2_t[g] = h
                r = rypool.tile([128, GROUP, D], F16, tag="ry")
                nc.scalar.activation(r[:], ps[0][:], RELU, bias=bia_sb[:, 1, 0:1])
                ry_t[(g, 1)] = r
                del ps12_t[g]

            def act_l2(g):
                ps = ps12_t[g]
                for oc in range(2):
                    r = rypool.tile([128, GROUP, D], F16, tag="ry")
                    nc.scalar.activation(
                        r[:], ps[oc][:], RELU, bias=bia_sb[:, 2, oc : oc + 1]
                    )
                    ry_t[(g, 2 + oc)] = r
                del ps12_t[g]

            def red(g, chunk):
                i0 = g * GROUP
                nc.vector.reduce_sum(
                    s_sb[:, chunk, i0 : i0 + GROUP],
                    ry_t[(g, chunk)][:],
                    axis=AX_X,
                )
                del ry_t[(g, chunk)]

            # ---------------- prologue ----------------
            dma_xq(0)
            dma_xq(1)
            dma_bg(0)
            dma_bg(1)
            build_z0(0)
            mm_l0(0)
            act_l0(0)
            build_z0(1)
            build_z12(0, 1)

            # ---------------- steady-state pipeline ----------------
            for i in range(N_GROUPS):
                if i + 2 < N_GROUPS:
                    dma_xq(i + 2)
                if i >= 1:
                    red(i - 1, 2)
                    red(i - 1, 3)
                mm_l12(i, 1)
                act_l1(i)
                build_z12(i, 2)
                if i + 1 < N_GROUPS:
                    mm_l0(i + 1)
                    act_l0(i + 1)
                red(i, 1)
                if i == 0:
                    red(0, 0)
                if i + 1 < N_GROUPS:
                    red(i + 1, 0)
                    build_z12(i + 1, 1)
                if i + 2 < N_GROUPS:
                    dma_bg(i + 2)
                mm_l12(i, 2)
                act_l2(i)
                if i + 2 < N_GROUPS:
                    build_z0(i + 2)

            red(N_GROUPS - 1, 2)
            red(N_GROUPS - 1, 3)

            # ---------------- final FC ----------------
            fc_ps = fcp.tile([1, B_CORE], F32, tag="fc")
            for c in range(4):
                nc.tensor.matmul(
                    fc_ps[:],
                    fcw_sb[:, c : c + 1],
                    s_sb[:, c, :],
                    start=(c == 0),
                    stop=(c == 3),
                )
            osb = consts.tile([1, B_CORE], F32, tag="osb")
            nc.scalar.activation(osb[:], fc_ps[:], IDENT, bias=fcb_sb[0:1, 0:1])
            nc.sync.dma_start(out[:], osb[:])

    _legalize_waits(nc)
    return nc


def _legalize_waits(nc, max_waits=1):
    """walrus codegen allows at most 2 semaphore waits per instruction; spill
    the excess onto NoOps injected just before the offender on the same
    engine (same-engine FIFO makes this ordering-equivalent)."""
    for bb in nc.main_func.blocks:
        insts = bb.instructions
        new_list = []
        changed = False
        for ins in insts:
            si = ins.sync_info
            if si is not None and si.on_wait and len(si.on_wait) > max_waits:
                waits = list(si.on_wait)
                extra, keep = waits[:-max_waits], waits[-max_waits:]
                k = 0
                while k < len(extra):
                    chunk = extra[k : k + max_waits]
                    nop = mybir.InstNoOp(name=f"{ins.name}-w{k}", ins=[], outs=[])
                    nop.engine = ins.engine
                    nop.sync_info = mybir.SyncInfo(on_wait=chunk, on_update=[])
                    new_list.append(nop)
                    k += max_waits
                ins.sync_info = mybir.SyncInfo(
                    on_wait=keep,
                    on_update=list(si.on_update) if si.on_update else [],
                )
                changed = True
            new_list.append(ins)
        if changed:
            if hasattr(bb, "set_instructions"):
                bb.set_instructions(new_list)
            else:
                insts.clear()
                insts.extend(new_list)
                if len(bb.instructions) != len(new_list):
                    bb.instructions = new_list


def _sym_maps():
    """amap/bmap: pair index k' -> (a, b) with a <= b, padded to C0*128."""
    a, b = np.triu_indices(M)
    pad = C0 * 128 - NP0
    amap = np.concatenate([a, np.zeros(pad, np.int64)])
    bmap = np.concatenate([b, np.zeros(pad, np.int64)])
    return amap, bmap


def prep_inputs(x, W0, b0, W1, b1, W2, b2, fc_W, fc_b):
    """Host-side reshape/cast into the per-core input maps."""
    xh = np.ascontiguousarray(x.astype(np.float16))
    amap, bmap = _sym_maps()
    # xqa[i, p, c, d] = xh[i, amap[c*128+p], d]
    idx_a = amap.reshape(C0, 128).T  # (128, C0)
    idx_b = bmap.reshape(C0, 128).T
    xqa = np.ascontiguousarray(xh[:, idx_a, :])
    xqb = np.ascontiguousarray(xh[:, idx_b, :])
    # fold W0 over symmetric pairs: rows a<b get W0[a,b]+W0[b,a]
    W0r = np.asarray(W0, np.float32).reshape(M, M, H)
    Wsym = W0r[amap[:NP0], bmap[:NP0]] + np.where(
        (amap[:NP0] != bmap[:NP0])[:, None], W0r[bmap[:NP0], amap[:NP0]], 0.0
    )
    Wpad = np.zeros((C0 * 128, H), np.float32)
    Wpad[:NP0] = Wsym
    w0s = np.ascontiguousarray(Wpad.astype(np.float16).reshape(C0, 128, H))
    w1 = np.ascontiguousarray(W1.astype(np.float16).reshape(32, 128, H))
    w2 = np.ascontiguousarray(W2.astype(np.float16).reshape(32, 128, H))
    bia = np.ascontiguousarray(
        np.stack([b0, b1, b2]).reshape(3, 2, 128).transpose(2, 0, 1).astype(np.float32)
    )
    fcw = np.ascontiguousarray(fc_W.reshape(4, 128).T.astype(np.float32))
    fcb = np.ascontiguousarray(fc_b.reshape(1, 1).astype(np.float32))
    shared = {"w0s": w0s, "w1": w1, "w2": w2, "bia": bia, "fcw": fcw, "fcb": fcb}
    return [
        {
            "xh": xh[i * B_CORE : (i + 1) * B_CORE],
            "xqa": xqa[i * B_CORE : (i + 1) * B_CORE],
            "xqb": xqb[i * B_CORE : (i + 1) * B_CORE],
            **shared,
        }
        for i in range(N_CORES)
    ]


_NC = None


def _get_nc():
    global _NC
    if _NC is None:
        _NC = build()
    return _NC


def kernel(**inputs):
    in_maps = prep_inputs(**inputs)
    res = run_bass_kernel_spmd(_get_nc(), in_maps, list(range(N_CORES)))
    return np.ascontiguousarray(
        np.concatenate([r["out"] for r in res.results], axis=0).astype(np.float32)
    )


# revision 4
# speedup vs baseline: 1.4060x; 1.0162x over previous
"""Compressed Interaction Network (CIN) forward on 8 Trainium2 NeuronCores.

Math (per batch item, m=32 fields, d=64 embed, H=256 hidden):
    x0 = x[i]                          # (m, d)
    h  = x0
    layer l in 0..2:
        z = outer(x0, h) over d        # (m*n, d), z[(a,b),:] = x0[a,:]*h[b,:]
        y = relu(W_l^T z + b_l)        # (H, d)
        xcur, h = split_half(y) (layers 0,1); xcur = h = y (layer 2)
    f = concat(xcur_0, xcur_1, xcur_2) # (512, d)
    out[i] = sum_d(f) @ fc_W + fc_b    # scalar

Mapping: batch 1024 -> 8 cores x 128 items, 16 groups of 8 items per core.

v2 design notes (vs the earlier per-group-serial version):
 - Layer 0 exploits z0 symmetry: z0[(a,b)] = z0[(b,a)], so W0 is folded on
   the host to 528 = 32*33/2 unique pair rows (padded to 640 = 5 k-chunks
   of 128).  The two z0 operands are host-gathered tensors (xqa, xqb) laid
   out so ONE full-128-partition DVE multiply builds a whole group's z0.
 - Layers 1/2: z chunks built on VectorE in fp16 (2x mode) from h x Bg
   (x broadcast to all partitions by one DMA per group); each chunk is
   consumed immediately by two interleaved matmuls (oc0/oc1) so at most a
   couple of chunks are alive.
 - Software pipeline over groups keeps the PE queue dense (HAM stays at
   K=8/8).  PE order per iteration i: [L1(i)][L0(i+1)][L2(i)].  The ACT of
   h2(i) + DVE build of z2(i) hide under L0(i+1); ACT h1(i+1) + build of
   z1(i+1) hide under L2(i).
 - Bias+ReLU fused in the PSUM->SBUF ACT; per-item d-sums for the final FC
   are 4 DVE X-axis reduces per group (fp32 out) instead of 512 per-item
   ScalarE accum ops.
 - Final dot: PE matmul of [128,1] fc weight chunks against [128,128] sums.
"""

import numpy as np

import concourse.bass as bass
import concourse.tile as tile
from concourse import mybir
from concourse.bass_utils import run_bass_kernel_spmd

N_CORES = 8
B_TOTAL = 1024
B_CORE = B_TOTAL // N_CORES  # 128
M = 32  # num fields
D = 64  # embed dim
H = 256  # conv output channels
GROUP = 8  # items per group (512 moving columns)
N_GROUPS = B_CORE // GROUP  # 16
MD = M * D  # 2048, elements per item row
NP0 = (M * (M + 1)) // 2  # 528 unique symmetric pairs in layer 0
C0 = (NP0 + 127) // 128  # 5 k-chunks of 128 (padded with zero weight rows)

F16 = mybir.dt.float16
F32 = mybir.dt.float32
RELU = mybir.ActivationFunctionType.Relu
IDENT = mybir.ActivationFunctionType.Identity
AX_X = mybir.AxisListType.X


def build():
    nc = bass.Bass()
    xh = nc.declare_dram_parameter("xh", [B_CORE, M, D], F16, isOutput=False)
    # layer-0 symmetric-pair operands: xqa[i, p, c, d] = x_i[amap[c*128+p], d]
    xqa = nc.declare_dram_parameter("xqa", [B_CORE, 128, C0, D], F16, isOutput=False)
    xqb = nc.declare_dram_parameter("xqb", [B_CORE, 128, C0, D], F16, isOutput=False)
    w0s = nc.declare_dram_parameter("w0s", [C0, 128, H], F16, isOutput=False)
    w1 = nc.declare_dram_parameter("w1", [32, 128, H], F16, isOutput=False)
    w2 = nc.declare_dram_parameter("w2", [32, 128, H], F16, isOutput=False)
    bia = nc.declare_dram_parameter("bia", [128, 3, 2], F32, isOutput=False)
    fcw = nc.declare_dram_parameter("fcw", [128, 4], F32, isOutput=False)
    fcb = nc.declare_dram_parameter("fcb", [1, 1], F32, isOutput=False)
    out = nc.declare_dram_parameter("out", [B_CORE, 1], F32, isOutput=True)

    with tile.TileContext(nc) as tc:
        with (
            tc.tile_pool(name="consts", bufs=1) as consts,
            tc.tile_pool(name="bgpool", bufs=2) as bgpool,
            tc.tile_pool(name="xqpool", bufs=2) as xqpool,
            tc.tile_pool(name="z0pool", bufs=3) as z0pool,
            tc.tile_pool(name="zpool", bufs=12) as zpool,
            tc.tile_pool(name="hpool", bufs=4) as hpool,
            tc.tile_pool(name="rypool", bufs=10) as rypool,
            tc.tile_pool(name="spool", bufs=1) as spool,
            tc.tile_pool(name="ppool", bufs=6, space="PSUM") as ppool,
            tc.tile_pool(name="fcp", bufs=1, space="PSUM") as fcp,
        ):
            # const tiles declared up front; DMAs are interleaved with the
            # group-0/1 input DMAs below in consumption order so the pipeline
            # can start ~40us earlier (the first z0/L0/L1 work doesn't sit
            # behind 8MB of weight traffic).
            w0s_sb = consts.tile([128, C0, H], F16, tag="w0s")
            w1_sb = consts.tile([128, 32, H], F16, tag="w1")
            w2_sb = consts.tile([128, 32, H], F16, tag="w2")
            bia_sb = consts.tile([128, 3, 2], F32, tag="bia")
            fcw_sb = consts.tile([128, 4], F32, tag="fcw")
            fcb_sb = consts.tile([1, 1], F32, tag="fcb")

            # per-item d-sums of the relu'd s-half channels, [chan, chunk, item]
            s_sb = spool.tile([128, 4, B_CORE], F32, tag="s")

            # pipeline state (python-side references to live tiles)
            bg_t, xqa_t, xqb_t = {}, {}, {}
            z0_t, z1_t, z2_t = {}, {}, {}
            h1_t, h2_t = {}, {}
            ry_t = {}  # (g, chunk) -> tile; chunks 0..3 = L0oc0, L1oc0, L2oc0, L2oc1
            ps01_t, ps12_t = {}, {}  # psum pairs for L0, and for L1/L2

            def dma_xq(g):
                i0 = g * GROUP
                for name, src, dst_map in (("xqa", xqa, xqa_t), ("xqb", xqb, xqb_t)):
                    t = xqpool.tile([128, C0, GROUP, D], F16, tag=name)
                    ap = bass.AP(
                        tensor=src,
                        offset=i0 * 128 * C0 * D,
                        ap=[[C0 * D, 128], [D, C0], [128 * C0 * D, GROUP], [1, D]],
                    )
                    nc.sync.dma_start(t[:], ap)
                    dst_map[g] = t

            def dma_bg(g):
                i0 = g * GROUP
                t = bgpool.tile([128, GROUP, M, D], F16, tag="B")
                src = bass.AP(
                    tensor=xh,
                    offset=i0 * MD,
                    ap=[[0, 128], [MD, GROUP], [1, MD]],
                )
                nc.sync.dma_start(t[:], src)
                bg_t[g] = t

            def build_z0(g):
                t = z0pool.tile([128, C0, GROUP, D], F16, tag="z0")
                nc.vector.tensor_mul(t[:], xqa_t[g][:], xqb_t[g][:])
                z0_t[g] = t
                del xqa_t[g], xqb_t[g]

            def mm_l0(g):
                ps = [
                    ppool.tile([128, GROUP * D], F32, tag="ps", name="ps0")
                    for _ in range(2)
                ]
                for c in range(C0):
                    for oc in range(2):
                        nc.tensor.matmul(
                            ps[oc][:],
                            w0s_sb[:, c, oc * 128 : (oc + 1) * 128],
                            z0_t[g][:, c, :, :],
                            start=(c == 0),
                            stop=(c == C0 - 1),
                        )
                ps01_t[g] = ps
                del z0_t[g]

            def act_l0(g):
                ps = ps01_t[g]
                h = hpool.tile([128, GROUP, D], F16, tag="h1")
                nc.scalar.activation(h[:], ps[1][:], RELU, bias=bia_sb[:, 0, 1:2])
                h1_t[g] = h
                r = rypool.tile([128, GROUP, D], F16, tag="ry")
                nc.scalar.activation(r[:], ps[0][:], RELU, bias=bia_sb[:, 0, 0:1])
                ry_t[(g, 0)] = r
                del ps01_t[g]

            def build_z12(g, lay):
                h = h1_t[g] if lay == 1 else h2_t[g]
                tiles = []
                for mb in range(8):
                    zt = zpool.tile([128, GROUP, 4, D], F16, tag="z")
                    nc.vector.tensor_mul(
                        zt[:],
                        h[:, :, None, :].to_broadcast((128, GROUP, 4, D)),
                        bg_t[g][:, :, 4 * mb : 4 * mb + 4, :],
                    )
                    tiles.append(zt)
                if lay == 1:
                    z1_t[g] = tiles
                    del h1_t[g]
                else:
                    z2_t[g] = tiles
                    del h2_t[g]

            def mm_l12(g, lay):
                w_sb = w1_sb if lay == 1 else w2_sb
                tiles = z1_t[g] if lay == 1 else z2_t[g]
                ps = [
                    ppool.tile([128, GROUP * D], F32, tag="ps", name="ps12")
                    for _ in range(2)
                ]
                for mb in range(8):
                    for mm in range(4):
                        m = 4 * mb + mm
                        for oc in range(2):
                            nc.tensor.matmul(
                                ps[oc][:],
                                w_sb[:, m, oc * 128 : (oc + 1) * 128],
                                tiles[mb][:, :, mm, :],
                                start=(m == 0),
                                stop=(m == 31),
                            )
                ps12_t[g] = ps
                if lay == 1:
                    del z1_t[g]
                else:
                    del z2_t[g]

            def act_l1(g):
                ps = ps12_t[g]
                h = hpool.tile([128, GROUP, D], F16, tag="h2")
                nc.scalar.activation(h[:], ps[1][:], RELU, bias=bia_sb[:, 1, 1:2])
                h2_t[g] = h
                r = rypool.tile([128, GROUP, D], F16, tag="ry")
                nc.scalar.activation(r[:], ps[0][:], RELU, bias=bia_sb[:, 1, 0:1])
                ry_t[(g, 1)] = r
                del ps12_t[g]

            def act_l2(g):
                ps = ps12_t[g]
                for oc in range(2):
                    r = rypool.tile([128, GROUP, D], F16, tag="ry")
                    nc.scalar.activation(
                        r[:], ps[oc][:], RELU, bias=bia_sb[:, 2, oc : oc + 1]
                    )
                    ry_t[(g, 2 + oc)] = r
                del ps12_t[g]

            def red(g, chunk):
                i0 = g * GROUP
                nc.vector.reduce_sum(
                    s_sb[:, chunk, i0 : i0 + GROUP],
                    ry_t[(g, chunk)][:],
                    axis=AX_X,
                )
                del ry_t[(g, chunk)]

            # ---------------- prologue ----------------
            # DMA issue order = consumption order of the startup chain.
            dma_xq(0)
            nc.sync.dma_start(w0s_sb[:], w0s[:].rearrange("c k o -> k c o"))
            nc.sync.dma_start(bia_sb[:], bia[:])
            dma_bg(0)
            nc.sync.dma_start(w1_sb[:], w1[:].rearrange("c k o -> k c o"))
            dma_xq(1)
            dma_bg(1)
            nc.sync.dma_start(w2_sb[:], w2[:].rearrange("c k o -> k c o"))
            nc.sync.dma_start(fcw_sb[:], fcw[:])
            nc.sync.dma_start(fcb_sb[:], fcb[:])
            build_z0(0)
            mm_l0(0)
            act_l0(0)
            build_z12(0, 1)
            build_z0(1)

            # ---------------- steady-state pipeline ----------------
            # PE order per iter: [L1(i)][L0(i+1)][L2(i)]; DVE order:
            # [z2(i)][z1(i+1)][z0(i+2)][reduces] (reduces last — they are
            # not on the z critical chain that feeds the PE).
            for i in range(N_GROUPS):
                if i + 2 < N_GROUPS:
                    dma_xq(i + 2)
                mm_l12(i, 1)
                act_l1(i)
                build_z12(i, 2)
                if i + 1 < N_GROUPS:
                    mm_l0(i + 1)
                    act_l0(i + 1)
                    build_z12(i + 1, 1)
                if i + 2 < N_GROUPS:
                    dma_bg(i + 2)
                mm_l12(i, 2)
                act_l2(i)
                if i + 2 < N_GROUPS:
                    build_z0(i + 2)
                if i >= 1:
                    red(i - 1, 2)
                    red(i - 1, 3)
                red(i, 1)
                if i == 0:
                    red(0, 0)
                if i + 1 < N_GROUPS:
                    red(i + 1, 0)

            red(N_GROUPS - 1, 2)
            red(N_GROUPS - 1, 3)

            # ---------------- final FC ----------------
            fc_ps = fcp.tile([1, B_CORE], F32, tag="fc")
            for c in range(4):
                nc.tensor.matmul(
                    fc_ps[:],
                    fcw_sb[:, c : c + 1],
                    s_sb[:, c, :],
                    start=(c == 0),
                    stop=(c == 3),
                )
            osb = consts.tile([1, B_CORE], F32, tag="osb")
            nc.scalar.activation(osb[:], fc_ps[:], IDENT, bias=fcb_sb[0:1, 0:1])
            nc.sync.dma_start(out[:], osb[:])

    _legalize_waits(nc)
    return nc


def _legalize_waits(nc, max_waits=1):
    """walrus codegen allows at most 2 semaphore waits per instruction; spill
    the excess onto NoOps injected just before the offender on the same
    engine (same-engine FIFO makes this ordering-equivalent)."""
    for bb in nc.main_func.blocks:
        insts = bb.instructions
        new_list = []
        changed = False
        for ins in insts:
            si = ins.sync_info
            if si is not None and si.on_wait and len(si.on_wait) > max_waits:
                waits = list(si.on_wait)
                extra, keep = waits[:-max_waits], waits[-max_waits:]
                k = 0
                while k < len(extra):
                    chunk = extra[k : k + max_waits]
                    nop = mybir.InstNoOp(name=f"{ins.name}-w{k}", ins=[], outs=[])
                    nop.engine = ins.engine
                    nop.sync_info = mybir.SyncInfo(on_wait=chunk, on_update=[])
                    new_list.append(nop)
                    k += max_waits
                ins.sync_info = mybir.SyncInfo(
                    on_wait=keep,
                    on_update=list(si.on_update) if si.on_update else [],
                )
                changed = True
            new_list.append(ins)
        if changed:
            if hasattr(bb, "set_instructions"):
                bb.set_instructions(new_list)
            else:
                insts.clear()
                insts.extend(new_list)
                if len(bb.instructions) != len(new_list):
                    bb.instructions = new_list


def _sym_maps():
    """amap/bmap: pair index k' -> (a, b) with a <= b, padded to C0*128."""
    a, b = np.triu_indices(M)
    pad = C0 * 128 - NP0
    amap = np.concatenate([a, np.zeros(pad, np.int64)])
    bmap = np.concatenate([b, np.zeros(pad, np.int64)])
    return amap, bmap


def prep_inputs(x, W0, b0, W1, b1, W2, b2, fc_W, fc_b):
    """Host-side reshape/cast into the per-core input maps."""
    xh = np.ascontiguousarray(x.astype(np.float16))
    amap, bmap = _sym_maps()
    # xqa[i, p, c, d] = xh[i, amap[c*128+p], d]
    idx_a = amap.reshape(C0, 128).T  # (128, C0)
    idx_b = bmap.reshape(C0, 128).T
    xqa = np.ascontiguousarray(xh[:, idx_a, :])
    xqb = np.ascontiguousarray(xh[:, idx_b, :])
    # fold W0 over symmetric pairs: rows a<b get W0[a,b]+W0[b,a]
    W0r = np.asarray(W0, np.float32).reshape(M, M, H)
    Wsym = W0r[amap[:NP0], bmap[:NP0]] + np.where(
        (amap[:NP0] != bmap[:NP0])[:, None], W0r[bmap[:NP0], amap[:NP0]], 0.0
    )
    Wpad = np.zeros((C0 * 128, H), np.float32)
    Wpad[:NP0] = Wsym
    w0s = np.ascontiguousarray(Wpad.astype(np.float16).reshape(C0, 128, H))
    w1 = np.ascontiguousarray(W1.astype(np.float16).reshape(32, 128, H))
    w2 = np.ascontiguousarray(W2.astype(np.float16).reshape(32, 128, H))
    bia = np.ascontiguousarray(
        np.stack([b0, b1, b2]).reshape(3, 2, 128).transpose(2, 0, 1).astype(np.float32)
    )
    fcw = np.ascontiguousarray(fc_W.reshape(4, 128).T.astype(np.float32))
    fcb = np.ascontiguousarray(fc_b.reshape(1, 1).astype(np.float32))
    shared = {"w0s": w0s, "w1": w1, "w2": w2, "bia": bia, "fcw": fcw, "fcb": fcb}
    return [
        {
            "xh": xh[i * B_CORE : (i + 1) * B_CORE],
            "xqa": xqa[i * B_CORE : (i + 1) * B_CORE],
            "xqb": xqb[i * B_CORE : (i + 1) * B_CORE],
            **shared,
        }
        for i in range(N_CORES)
    ]


_NC = None


def _get_nc():
    global _NC
    if _NC is None:
        _NC = build()
    return _NC


def kernel(**inputs):
    in_maps = prep_inputs(**inputs)
    res = run_bass_kernel_spmd(_get_nc(), in_maps, list(range(N_CORES)))
    return np.ascontiguousarray(
        np.concatenate([r["out"] for r in res.results], axis=0).astype(np.float32)
    )
